# revision 32
# baseline (speedup 1.0000x reference)
"""Distributed causal multi-head attention (GPT-2 style block) for one TRN2 chip.

Sharding over 8 NeuronCores: core c -> (batch b = c//2, head-group g = c%2).
Each core computes QKV for its batch restricted to its 8 heads (tensor-
parallel column split of W_attn), runs causal attention for those heads,
computes a full-width partial of the output projection from its local 512
head features, and the pair of cores sharing a batch ReduceScatter-sums the
partials (each keeps its 512-output-feature shard). Host assembles the full
[4, 2048, 1024] output.

Pipelining: the attention inner loop is ACT(exp)-paced; QKV matmul units for
the NEXT sequence chunk and output-projection units for chunk qc-1 are
injected between attention score/PV groups as PE filler work, so no PE work
ever waits on a collective. The ReduceScatter result is converted bf16->f32
and written to the output by a GPSIMD cast-DMA (its own SWDGE queue), riding
two chunks behind.

DMA layout: bulk loads (weights, x chunks) are single coalesced descriptors
on the ACT HWDGE queue; attention-side stores ride the SP queue; the output
conversion rides the Pool SWDGE queue. The V ones-column (softmax
denominator trick) is memset on-chip rather than DMA'd (a scatter DMA of
16K 2-byte descriptors would serialize the queue for ~50us).

Matmul dtypes: f32r (full-rate fp32, ~1e-4 rel err) for QKV + scores,
bf16 for exp(P)/V and the output projection. Softmax runs without
max-subtraction (logits are bounded), with the denominator computed by
augmenting V with a ones column so P@[V|1] yields numerator + denominator.
"""
import numpy as np
import ml_dtypes

B, S, D = 4, 2048, 1024
H, HD, HPC = 16, 64, 8
DL = HPC * HD            # 512 local head features per core
P = 128
CW = 512                 # q-chunk width
NQC = S // CW            # 4
NKT = S // P             # 16
KC = D // P              # 8 contraction chunks of 128
GRP = 2                  # k-tiles per score/exp group (2 PSUM banks)
VW = 65                  # per-head V width incl. ones column
MW = 384 + CW            # compacted causal mask width

_CACHE: dict = {}


def _build(debug=False, ablate=None):
    from concourse import bacc
    import concourse.mybir as mybir
    from concourse.tile import TileContext

    F32, F32R, BF16 = mybir.dt.float32, mybir.dt.float32r, mybir.dt.bfloat16
    AF = mybir.ActivationFunctionType
    ALU = mybir.AluOpType

    nc = bacc.Bacc(trn_type="TRN2", num_devices=8)
    if debug:
        dbg_qk = nc.declare_dram_parameter("dbg_qk", [P, 8, S], F32R, isOutput=True)
        dbg_vp = nc.declare_dram_parameter("dbg_vp", [P, NKT, HPC * VW], BF16, isOutput=True)
        dbg_rc = nc.declare_dram_parameter("dbg_rc", [NQC * HPC, CW], F32, isOutput=True)
    xT = nc.declare_dram_parameter("xT", [D, S], F32R, isOutput=False)
    wqkv = nc.declare_dram_parameter("wqkv", [D, 3 * DL], F32R, isOutput=False)
    bqk = nc.declare_dram_parameter("bqk", [P, 8], F32, isOutput=False)
    bv = nc.declare_dram_parameter("bv", [1, DL], F32, isOutput=False)
    wp = nc.declare_dram_parameter("wp", [DL, D], BF16, isOutput=False)
    bp = nc.declare_dram_parameter("bp", [P, 8], F32, isOutput=False)
    maskc = nc.declare_dram_parameter("maskc", [P, MW], BF16, isOutput=False)
    out_ext = nc.declare_dram_parameter("out", [DL, S], F32, isOutput=True)

    rs_in = nc.dram_tensor("rs_in", [NQC, 2 * DL, CW], BF16)
    rs_out = nc.dram_tensor("rs_out", [NQC, DL, CW], BF16)
    RG = [[0, 1], [2, 3], [4, 5], [6, 7]]

    with TileContext(nc) as tc:
        with tc.tile_pool(name="const", bufs=1) as constp, \
             tc.tile_pool(name="persist", bufs=1) as perp, \
             tc.tile_pool(name="wq", bufs=1) as wqp, \
             tc.tile_pool(name="xt", bufs=2) as xtp, \
             tc.tile_pool(name="qtp", bufs=2) as qtp, \
             tc.tile_pool(name="wpp", bufs=1) as wpp, \
             tc.tile_pool(name="ptp", bufs=3) as ptp, \
             tc.tile_pool(name="atp", bufs=3) as atp, \
             tc.tile_pool(name="smallp", bufs=2) as smallp, \
             tc.tile_pool(name="denp", bufs=1) as denp, \
             tc.tile_pool(name="popp", bufs=2) as popp, \
             tc.tile_pool(name="cvb", bufs=2) as cvbp, \
             tc.tile_pool(name="otp", bufs=2) as otp, \
             tc.tile_pool(name="ps3", bufs=2, space="PSUM") as ps3, \
             tc.tile_pool(name="ps1", bufs=2, space="PSUM") as ps1, \
             tc.tile_pool(name="psq", bufs=2, space="PSUM") as psq:

            # ---- constants ----
            bqk_t = constp.tile([P, 8], F32)
            nc.sync.dma_start(out=bqk_t[:], in_=bqk[:])
            bp_t = constp.tile([P, 8], F32)
            nc.sync.dma_start(out=bp_t[:], in_=bp[:])
            maskr = constp.tile([P, MW], BF16)
            nc.sync.dma_start(out=maskr[:], in_=maskc[:])
            bv_stage = constp.tile([1, DL], F32)
            nc.sync.dma_start(out=bv_stage[:], in_=bv[:])
            bias_bc = constp.tile([P, DL], F32)
            nc.gpsimd.partition_broadcast(bias_bc[:], bv_stage[:])

            # ---- long-lived activations ----
            k_all = perp.tile([P, 4, S], F32R)
            vpad = perp.tile([P, NKT, HPC * VW], BF16)   # v + ones col per head
            nc.vector.memset(
                vpad[:].rearrange("p nk (h c) -> p (nk h) c", c=VW)[:, :, HD:VW],
                1.0)

            # ---- weights on the ACT HWDGE queue, in consumption order:
            # v columns (split so chunk-0 V matmuls start as data arrives),
            # then q, k, and the projection weights.
            wq_t = wqp.tile([P, KC, 3 * DL], F32R)
            wq_src = wqkv[:].rearrange("(kc p) c -> p kc c", p=P)
            for h0 in range(0, KC, 2):
                nc.scalar.dma_start(
                    out=wq_t[:, h0:h0 + 2, 2 * DL:3 * DL],
                    in_=wq_src[:, h0:h0 + 2, 2 * DL:3 * DL])
            nc.scalar.dma_start(out=wq_t[:, :, 0:DL],
                                in_=wq_src[:, :, 0:DL])
            nc.scalar.dma_start(out=wq_t[:, :, DL:2 * DL],
                                in_=wq_src[:, :, DL:2 * DL])
            wp_t = wpp.tile([P, 4, D], BF16)
            nc.scalar.dma_start(out=wp_t[:],
                                in_=wp[:].rearrange("(kc p) c -> p kc c", p=P))

            qt_tiles = {}
            at_sets = {}
            xT_src = xT[:].rearrange("(kc p) s -> p kc s", p=P)

            def qkv_units(qc):
                """One generator item = one PE unit (8 matmuls + eviction)."""
                xtr = xtp.tile([P, KC, CW], F32R, tag="xtr",
                               name=f"xtr_{qc}")
                if qc == 0:
                    # SP queue (free at start) in kc-pair pieces so the first
                    # matmuls chase the arriving data
                    for h0 in range(0, KC, 2):
                        nc.sync.dma_start(
                            out=xtr[:, h0:h0 + 2, :],
                            in_=xT_src[:, h0:h0 + 2,
                                       qc * CW:(qc + 1) * CW])
                else:
                    nc.scalar.dma_start(out=xtr[:],
                                        in_=xT_src[:, :, qc * CW:(qc + 1) * CW])
                qt = qtp.tile([P, 4, CW], F32R, tag="qt", name=f"qt_{qc}")
                qt_tiles[qc] = qt

                def v_unit(stl):
                    pt = psq.tile([P, CW], F32, tag="psq", name=f"v_{qc}_{stl}")
                    for kc in range(KC):
                        nc.tensor.matmul(
                            out=pt[:],
                            lhsT=xtr[:, kc, stl * P:(stl + 1) * P],
                            rhs=wq_t[:, kc, 2 * DL:3 * DL],
                            start=(kc == 0), stop=(kc == KC - 1))
                    st = qc * 4 + stl
                    nc.vector.tensor_tensor(
                        out=vpad[:, st, :].rearrange(
                            "p (h c) -> p h c", c=VW)[:, :, 0:HD],
                        in0=pt[:].rearrange("p (h c) -> p h c", c=HD),
                        in1=bias_bc[:].rearrange("p (h c) -> p h c", c=HD),
                        op=ALU.add)

                def qk_unit(m):
                    pt = psq.tile([P, CW], F32, tag="psq", name=f"qk_{qc}_{m}")
                    for kc in range(KC):
                        nc.tensor.matmul(
                            out=pt[:],
                            lhsT=wq_t[:, kc, m * P:(m + 1) * P],
                            rhs=xtr[:, kc, :],
                            start=(kc == 0), stop=(kc == KC - 1))
                    dst = (qt[:, m, :] if m < 4
                           else k_all[:, m - 4, qc * CW:(qc + 1) * CW])
                    nc.vector.tensor_scalar_add(
                        out=dst, in0=pt[:], scalar1=bqk_t[:, m:m + 1])

                for stl in range(4):
                    yield lambda stl=stl: v_unit(stl)
                for m in range(8):
                    yield lambda m=m: qk_unit(m)

            def proj_units(qc):
                """Local partial projection: 4 od-pair units (8 matmuls each),
                each half's ReduceScatter issued as soon as its two od-pairs
                are out. No matmul waits on any collective."""
                at4 = at_sets[qc]
                rs_dst = rs_in[qc].rearrange("(od p) cw -> p od cw", p=P)

                def unit(odp):
                    po = popp.tile([P, 2, CW], BF16, tag="po",
                                   name=f"po_{qc}_{odp}")
                    for j in range(2):
                        od = 2 * odp + j
                        pp = psq.tile([P, CW], F32, tag="psq",
                                      name=f"pp_{qc}_{od}")
                        for kc in range(4):
                            nc.tensor.matmul(
                                out=pp[:],
                                lhsT=wp_t[:, kc, od * P:(od + 1) * P],
                                rhs=at4[kc][:],
                                start=(kc == 0), stop=(kc == 3))
                        nc.vector.tensor_scalar_add(
                            out=po[:, j, :], in0=pp[:],
                            scalar1=bp_t[:, od:od + 1])
                    nc.sync.dma_start(
                        out=rs_dst[:, 2 * odp:2 * odp + 2, :], in_=po[:])

                def rs():
                    nc.gpsimd.collective_compute(
                        "ReduceScatter", ALU.add, replica_groups=RG,
                        ins=[rs_in[qc]], outs=[rs_out[qc]])

                for odp in range(4):
                    yield lambda odp=odp: unit(odp)
                yield rs

            def conv_units(qc, split=False):
                """bf16 ReduceScatter result -> f32 output rows via a single
                GPSIMD cast-DMA (SWDGE queue, DRAM->DRAM)."""
                def unit(r0, r1):
                    nc.gpsimd.dma_start(
                        out=out_ext[r0:r1, qc * CW:(qc + 1) * CW],
                        in_=rs_out[qc][r0:r1, :])
                if split:
                    yield lambda: unit(0, 256)
                    yield lambda: unit(256, DL)
                else:
                    yield lambda: unit(0, DL)

            fillers = []

            def drain_fillers():
                while fillers:
                    fillers.pop(0)()

            def emit_attention(qc):
                qt = qt_tiles[qc]
                at_tiles = [None] * 4
                at_sets[qc] = at_tiles
                # pace filler pops evenly across the chunk's score groups so
                # PE filler work spans the whole chunk (incl. the boundary)
                # instead of bunching at the front and leaving an ACT-paced
                # stretch that lets the HAM clock-gate drop to half rate
                ngroups = HPC * ((4 * (qc + 1) + GRP - 1) // GRP)
                n0 = len(fillers)
                state = {"slot": 0, "popped": 0}

                def pop_filler():
                    state["slot"] += 1
                    target = (state["slot"] * n0) // (ngroups + 4)
                    while state["popped"] < min(n0, target):
                        fillers.pop(0)()
                        state["popped"] += 1

                for h in range(HPC):
                    half = 64 * (h % 2)
                    qs = qt[half:half + 64, h // 2, :]
                    pa = ps1.tile([P, CW], F32, tag="pacc", name=f"pa_{qc}_{h}")
                    kmax = 4 * (qc + 1)
                    groups = [list(range(s, min(s + GRP, kmax)))
                              for s in range(0, kmax, GRP)]
                    pending = None  # (group, ptile)

                    def flush(pending, kmax=kmax, pa=pa, h=h, qc=qc):
                        g, ptile = pending
                        for j, kt in enumerate(g):
                            if kt >= 4 * qc:
                                pat = kt - 4 * qc
                                off = 384 - pat * P
                                nc.vector.tensor_mul(
                                    out=ptile[:, j * CW:(j + 1) * CW],
                                    in0=ptile[:, j * CW:(j + 1) * CW],
                                    in1=maskr[:, off:off + CW])
                        for j, kt in enumerate(g):
                            nc.tensor.matmul(
                                out=pa[0:VW, :],
                                lhsT=vpad[:, kt, h * VW:(h + 1) * VW],
                                rhs=ptile[:, j * CW:(j + 1) * CW],
                                start=(kt == 0), stop=(kt == kmax - 1))

                    for g in groups:
                        pt = ps3.tile([P, GRP * CW], F32, tag="ps3",
                                      name=f"sc_{qc}_{h}_{g[0]}")
                        for j, kt in enumerate(g):
                            nc.tensor.matmul(
                                out=pt[:, j * CW:(j + 1) * CW],
                                lhsT=k_all[half:half + 64, h // 2,
                                           kt * P:(kt + 1) * P],
                                rhs=qs, start=True, stop=True)
                        if pending is not None:
                            flush(pending)
                        w = len(g) * CW
                        ptile = ptp.tile([P, GRP * CW], BF16, tag="pt",
                                         name=f"pt_{qc}_{h}_{g[0]}")
                        nc.scalar.activation(ptile[:, :w], pt[:, :w],
                                             AF.Exp, scale=0.125)
                        pending = (g, ptile)
                        pop_filler()
                    flush(pending)

                    # normalize by the ones-row denominator
                    den = denp.tile([1, CW], F32, tag="den", name=f"den_{qc}_{h}")
                    nc.vector.tensor_copy(out=den[:], in_=pa[64:65, :])
                    rc = smallp.tile([1, CW], F32, tag="recip",
                                     name=f"rc_{qc}_{h}")
                    nc.vector.reciprocal_approx_fast(out=rc[:], in_=den[:])
                    if debug:
                        nc.sync.dma_start(
                            out=dbg_rc[qc * HPC + h:qc * HPC + h + 1, :],
                            in_=rc[:])
                    bc = smallp.tile([64, CW], F32, tag="bcast",
                                     name=f"bc_{qc}_{h}")
                    nc.gpsimd.partition_broadcast(bc[:], rc[:])
                    if h % 2 == 0:
                        at_tiles[h // 2] = atp.tile(
                            [P, CW], BF16, tag=f"at{h // 2}",
                            name=f"at_{qc}_{h // 2}")
                    nc.vector.tensor_tensor(
                        out=at_tiles[h // 2][half:half + 64, :],
                        in0=pa[0:64, :], in1=bc[:], op=ALU.mult)

            # ---- main schedule ----
            # chunk 0 QKV up front; later chunks' QKV and deferred
            # projections ride as PE fillers inside attention, paced so
            # every chunk (especially the long last one) stays PE-dense.
            for u in qkv_units(0):
                u()
            for qc in range(NQC):
                if ablate == "qkv":
                    if qc + 1 < NQC:
                        for u in qkv_units(qc + 1):
                            u()
                    continue
                if ablate != "attn":
                    if qc == 2:
                        fillers.extend(proj_units(0))
                    if qc == 3:
                        fillers.extend(proj_units(1))
                        fillers.extend(proj_units(2))
                if qc + 1 < NQC:
                    fillers.extend(qkv_units(qc + 1))
                emit_attention(qc)
                drain_fillers()
            if ablate is None:
                # all output conversions at the tail: their RS inputs are
                # long done (0-2) so they never block the GPSIMD queue
                # mid-kernel; rs3 triggers right after conv0's cheap issue
                for u in conv_units(0):
                    u()
                for u in proj_units(NQC - 1):
                    u()
                for j in (1, 2, 3):
                    for u in conv_units(j):
                        u()
            if debug:
                for qc in range(NQC):
                    for m in range(4):
                        nc.sync.dma_start(
                            out=dbg_qk[:, m, qc * CW:(qc + 1) * CW],
                            in_=qt_tiles[qc][:, m, :])
                for m in range(4):
                    nc.sync.dma_start(out=dbg_qk[:, 4 + m, :],
                                      in_=k_all[:, m, :])
                for st in range(NKT):
                    nc.sync.dma_start(out=dbg_vp[:, st, :], in_=vpad[:, st, :])

    nc.finalize()
    return nc


def _get_nc():
    if "nc" not in _CACHE:
        _CACHE["nc"] = _build()
    return _CACHE["nc"]


def _make_mask():
    # M[p, c] = 1.0 iff (c - 384) >= p; pattern pat slice = cols [384-128*pat:][:CW]
    c = np.arange(MW)[None, :]
    p = np.arange(P)[:, None]
    return ((c - 384) >= p).astype(ml_dtypes.bfloat16)


def make_in_maps(x, W_attn, b_attn, W_proj, b_proj):
    x = np.asarray(x, np.float32)
    W_attn = np.asarray(W_attn, np.float32)
    b_attn = np.asarray(b_attn, np.float32)
    W_proj = np.asarray(W_proj, np.float32)
    b_proj = np.asarray(b_proj, np.float32)
    mask = _make_mask()
    in_maps = []
    for c in range(8):
        b, g = c // 2, c % 2
        sl = slice(g * DL, (g + 1) * DL)
        wqkv_c = np.concatenate([W_attn[:, g * DL:(g + 1) * DL],
                                 W_attn[:, D + g * DL:D + (g + 1) * DL],
                                 W_attn[:, 2 * D + g * DL:2 * D + (g + 1) * DL]],
                                axis=1)
        bqk_c = np.concatenate([b_attn[g * DL:(g + 1) * DL],
                                b_attn[D + g * DL:D + (g + 1) * DL]])
        in_maps.append({
            "xT": np.ascontiguousarray(x[b].T),
            "wqkv": np.ascontiguousarray(wqkv_c),
            "bqk": np.ascontiguousarray(bqk_c.reshape(8, P).T),
            "bv": b_attn[2 * D + g * DL:2 * D + (g + 1) * DL].reshape(1, DL).copy(),
            "wp": np.ascontiguousarray(W_proj[sl, :]).astype(ml_dtypes.bfloat16),
            "bp": (np.ascontiguousarray(b_proj.reshape(8, P).T) if g == 0
                   else np.zeros((P, 8), np.float32)),
            "maskc": mask,
        })
    return in_maps


def assemble(results):
    out = np.empty((B, S, D), np.float32)
    for c in range(8):
        b, g = c // 2, c % 2
        out[b][:, g * DL:(g + 1) * DL] = results[c]["out"].T
    return out


def kernel(x, W_attn, b_attn, W_proj, b_proj):
    from concourse.bass_utils import run_bass_kernel_spmd
    nc = _get_nc()
    in_maps = make_in_maps(x, W_attn, b_attn, W_proj, b_proj)
    res = run_bass_kernel_spmd(nc, in_maps, core_ids=list(range(8)))
    return assemble(res.results)


# revision 39
# speedup vs baseline: 1.1557x; 1.1557x over previous
"""Distributed causal multi-head attention (GPT-2 style block) for one TRN2 chip.

Sharding over 8 NeuronCores: core c -> (batch b = c//2, head-group g = c%2).
Each core computes QKV for its batch restricted to its 8 heads (tensor-
parallel column split of W_attn), runs causal attention for those heads,
computes a full-width partial of the output projection from its local 512
head features, and the pair of cores sharing a batch ReduceScatter-sums the
partials (each keeps its 512-output-feature shard). Host assembles the full
[4, 2048, 1024] output.

Pipelining: the attention inner loop is ACT(exp)-paced; QKV matmul units for
the NEXT sequence chunk and output-projection units for chunk qc-1 are
injected between attention score/PV groups as PE filler work, so no PE work
ever waits on a collective. The ReduceScatter result is converted bf16->f32
and written to the output by a GPSIMD cast-DMA (its own SWDGE queue), riding
two chunks behind.

DMA layout: bulk loads (weights, x chunks) are single coalesced descriptors
on the ACT HWDGE queue; attention-side stores ride the SP queue; the output
conversion rides the Pool SWDGE queue. The V ones-column (softmax
denominator trick) is memset on-chip rather than DMA'd (a scatter DMA of
16K 2-byte descriptors would serialize the queue for ~50us).

Matmul dtypes: f32r (full-rate fp32, ~1e-4 rel err) for QKV + scores,
bf16 for exp(P)/V and the output projection. Softmax runs without
max-subtraction (logits are bounded), with the denominator computed by
augmenting V with a ones column so P@[V|1] yields numerator + denominator.
"""
import numpy as np
import ml_dtypes

B, S, D = 4, 2048, 1024
H, HD, HPC = 16, 64, 8
DL = HPC * HD            # 512 local head features per core
P = 128
CW = 512                 # q-chunk width
NQC = S // CW            # 4
NKT = S // P             # 16
KC = D // P              # 8 contraction chunks of 128
GRP = 2                  # k-tiles per score/exp group (2 PSUM banks)
VW = 65                  # per-head V width incl. ones column
MW = 384 + CW            # compacted causal mask width

_CACHE: dict = {}


def _build(debug=False, ablate=None):
    from concourse import bacc
    import concourse.mybir as mybir
    from concourse.tile import TileContext

    F32, F32R, BF16 = mybir.dt.float32, mybir.dt.float32r, mybir.dt.bfloat16
    AF = mybir.ActivationFunctionType
    ALU = mybir.AluOpType

    nc = bacc.Bacc(trn_type="TRN2", num_devices=8)
    if debug:
        dbg_qk = nc.declare_dram_parameter("dbg_qk", [P, 8, S], F32R, isOutput=True)
        dbg_vp = nc.declare_dram_parameter("dbg_vp", [P, NKT, HPC * VW], BF16, isOutput=True)
        dbg_rc = nc.declare_dram_parameter("dbg_rc", [NQC * HPC, CW], F32, isOutput=True)
    xT = nc.declare_dram_parameter("xT", [D, S], F32R, isOutput=False)
    wqkv = nc.declare_dram_parameter("wqkv", [D, 3 * DL], F32R, isOutput=False)
    bqk = nc.declare_dram_parameter("bqk", [P, 8], F32, isOutput=False)
    bv = nc.declare_dram_parameter("bv", [1, DL], F32, isOutput=False)
    wp = nc.declare_dram_parameter("wp", [DL, D], BF16, isOutput=False)
    bp = nc.declare_dram_parameter("bp", [P, 8], F32, isOutput=False)
    maskc = nc.declare_dram_parameter("maskc", [P, MW], BF16, isOutput=False)
    out_ext = nc.declare_dram_parameter("out", [DL, S], F32, isOutput=True)

    rs_in = nc.dram_tensor("rs_in", [NQC, 2 * DL, CW], BF16)
    rs_out = nc.dram_tensor("rs_out", [NQC, DL, CW], BF16)
    RG = [[0, 1], [2, 3], [4, 5], [6, 7]]

    with TileContext(nc) as tc:
        with tc.tile_pool(name="const", bufs=1) as constp, \
             tc.tile_pool(name="persist", bufs=1) as perp, \
             tc.tile_pool(name="wq", bufs=1) as wqp, \
             tc.tile_pool(name="xt", bufs=2) as xtp, \
             tc.tile_pool(name="qtp", bufs=2) as qtp, \
             tc.tile_pool(name="wpp", bufs=1) as wpp, \
             tc.tile_pool(name="ptp", bufs=3) as ptp, \
             tc.tile_pool(name="atp", bufs=3) as atp, \
             tc.tile_pool(name="smallp", bufs=2) as smallp, \
             tc.tile_pool(name="denp", bufs=1) as denp, \
             tc.tile_pool(name="popp", bufs=2) as popp, \
             tc.tile_pool(name="cvb", bufs=2) as cvbp, \
             tc.tile_pool(name="otp", bufs=2) as otp, \
             tc.tile_pool(name="ps3", bufs=2, space="PSUM") as ps3, \
             tc.tile_pool(name="ps1", bufs=2, space="PSUM") as ps1, \
             tc.tile_pool(name="psq", bufs=2, space="PSUM") as psq:

            # ---- constants ----
            bqk_t = constp.tile([P, 8], F32)
            nc.sync.dma_start(out=bqk_t[:], in_=bqk[:])
            bp_t = constp.tile([P, 8], F32)
            nc.sync.dma_start(out=bp_t[:], in_=bp[:])
            maskr = constp.tile([P, MW], BF16)
            nc.sync.dma_start(out=maskr[:], in_=maskc[:])
            bv_stage = constp.tile([1, DL], F32)
            nc.sync.dma_start(out=bv_stage[:], in_=bv[:])
            bias_bc = constp.tile([P, DL], F32)
            nc.gpsimd.partition_broadcast(bias_bc[:], bv_stage[:])
            ones64 = constp.tile([1, 64], BF16)
            nc.vector.memset(ones64[:], 1.0)

            # ---- long-lived activations ----
            k_all = perp.tile([P, 4, S], F32R)
            vpad = perp.tile([P, NKT, HPC * VW], BF16)   # v + ones col per head
            nc.vector.memset(
                vpad[:].rearrange("p nk (h c) -> p (nk h) c", c=VW)[:, :, HD:VW],
                1.0)

            # ---- weights on the ACT HWDGE queue, in consumption order:
            # v columns (split so chunk-0 V matmuls start as data arrives),
            # then q, k, and the projection weights.
            wq_t = wqp.tile([P, KC, 3 * DL], F32R)
            wq_src = wqkv[:].rearrange("(kc p) c -> p kc c", p=P)
            for h0 in range(0, KC, 2):
                nc.scalar.dma_start(
                    out=wq_t[:, h0:h0 + 2, 2 * DL:3 * DL],
                    in_=wq_src[:, h0:h0 + 2, 2 * DL:3 * DL])
            nc.scalar.dma_start(out=wq_t[:, :, 0:DL],
                                in_=wq_src[:, :, 0:DL])
            nc.scalar.dma_start(out=wq_t[:, :, DL:2 * DL],
                                in_=wq_src[:, :, DL:2 * DL])
            wp_t = wpp.tile([P, 4, D], BF16)
            nc.scalar.dma_start(out=wp_t[:],
                                in_=wp[:].rearrange("(kc p) c -> p kc c", p=P))

            qt_tiles = {}
            at_sets = {}
            xT_src = xT[:].rearrange("(kc p) s -> p kc s", p=P)

            def qkv_units(qc):
                """One generator item = one PE unit (8 matmuls + eviction)."""
                xtr = xtp.tile([P, KC, CW], F32R, tag="xtr",
                               name=f"xtr_{qc}")
                if qc == 0:
                    # SP queue (free at start) in kc-pair pieces so the first
                    # matmuls chase the arriving data
                    for h0 in range(0, KC, 2):
                        nc.sync.dma_start(
                            out=xtr[:, h0:h0 + 2, :],
                            in_=xT_src[:, h0:h0 + 2,
                                       qc * CW:(qc + 1) * CW])
                else:
                    nc.scalar.dma_start(out=xtr[:],
                                        in_=xT_src[:, :, qc * CW:(qc + 1) * CW])
                qt = qtp.tile([P, 4, CW], F32R, tag="qt", name=f"qt_{qc}")
                qt_tiles[qc] = qt

                def v_unit(stl):
                    pt = psq.tile([P, CW], F32, tag="psq", name=f"v_{qc}_{stl}")
                    for kc in range(KC):
                        nc.tensor.matmul(
                            out=pt[:],
                            lhsT=xtr[:, kc, stl * P:(stl + 1) * P],
                            rhs=wq_t[:, kc, 2 * DL:3 * DL],
                            start=(kc == 0), stop=(kc == KC - 1))
                    st = qc * 4 + stl
                    nc.vector.tensor_tensor(
                        out=vpad[:, st, :].rearrange(
                            "p (h c) -> p h c", c=VW)[:, :, 0:HD],
                        in0=pt[:].rearrange("p (h c) -> p h c", c=HD),
                        in1=bias_bc[:].rearrange("p (h c) -> p h c", c=HD),
                        op=ALU.add)

                def qk_unit(m):
                    pt = psq.tile([P, CW], F32, tag="psq", name=f"qk_{qc}_{m}")
                    for kc in range(KC):
                        nc.tensor.matmul(
                            out=pt[:],
                            lhsT=wq_t[:, kc, m * P:(m + 1) * P],
                            rhs=xtr[:, kc, :],
                            start=(kc == 0), stop=(kc == KC - 1))
                    dst = (qt[:, m, :] if m < 4
                           else k_all[:, m - 4, qc * CW:(qc + 1) * CW])
                    nc.vector.tensor_scalar_add(
                        out=dst, in0=pt[:], scalar1=bqk_t[:, m:m + 1])

                for stl in range(4):
                    yield lambda stl=stl: v_unit(stl)
                for m in range(8):
                    yield lambda m=m: qk_unit(m)

            def proj_units(qc):
                """Local partial projection: 4 od-pair units (8 matmuls each),
                each half's ReduceScatter issued as soon as its two od-pairs
                are out. No matmul waits on any collective."""
                at4 = at_sets[qc]
                rs_dst = rs_in[qc].rearrange("(od p) cw -> p od cw", p=P)

                def unit(odp):
                    po = popp.tile([P, 2, CW], BF16, tag="po",
                                   name=f"po_{qc}_{odp}")
                    for j in range(2):
                        od = 2 * odp + j
                        pp = psq.tile([P, CW], F32, tag="psq",
                                      name=f"pp_{qc}_{od}")
                        for kc in range(4):
                            nc.tensor.matmul(
                                out=pp[:],
                                lhsT=wp_t[:, kc, od * P:(od + 1) * P],
                                rhs=at4[kc][:],
                                start=(kc == 0), stop=(kc == 3))
                        nc.vector.tensor_scalar_add(
                            out=po[:, j, :], in0=pp[:],
                            scalar1=bp_t[:, od:od + 1])
                    nc.sync.dma_start(
                        out=rs_dst[:, 2 * odp:2 * odp + 2, :], in_=po[:])

                def rs():
                    nc.gpsimd.collective_compute(
                        "ReduceScatter", ALU.add, replica_groups=RG,
                        ins=[rs_in[qc]], outs=[rs_out[qc]])

                for odp in range(4):
                    yield lambda odp=odp: unit(odp)
                yield rs

            def conv_units(qc, split=False):
                """bf16 ReduceScatter result -> f32 output rows via a single
                GPSIMD cast-DMA (SWDGE queue, DRAM->DRAM)."""
                def unit(r0, r1):
                    nc.gpsimd.dma_start(
                        out=out_ext[r0:r1, qc * CW:(qc + 1) * CW],
                        in_=rs_out[qc][r0:r1, :])
                if split:
                    yield lambda: unit(0, 256)
                    yield lambda: unit(256, DL)
                else:
                    yield lambda: unit(0, DL)

            fillers = []

            def drain_fillers():
                while fillers:
                    fillers.pop(0)()

            def emit_attention(qc):
                qt = qt_tiles[qc]
                at_tiles = [None] * 4
                at_sets[qc] = at_tiles
                # pace filler pops evenly across the chunk's score groups so
                # PE filler work spans the whole chunk (incl. the boundary)
                # instead of bunching at the front and leaving an ACT-paced
                # stretch that lets the HAM clock-gate drop to half rate
                ngroups = HPC * ((4 * (qc + 1) + GRP - 1) // GRP)
                n0 = len(fillers)
                state = {"slot": 0, "popped": 0}

                def pop_filler():
                    state["slot"] += 1
                    target = (state["slot"] * n0) // (ngroups + 4)
                    while state["popped"] < min(n0, target):
                        fillers.pop(0)()
                        state["popped"] += 1

                for h in range(HPC):
                    half = 64 * (h % 2)
                    qs = qt[half:half + 64, h // 2, :]
                    pa = ps1.tile([P, CW], F32, tag="pacc", name=f"pa_{qc}_{h}")
                    kmax = 4 * (qc + 1)
                    groups = [list(range(s, min(s + GRP, kmax)))
                              for s in range(0, kmax, GRP)]
                    pending = None  # (group, ptile)

                    def flush(pending, kmax=kmax, pa=pa, h=h, qc=qc):
                        g, ptile = pending
                        for j, kt in enumerate(g):
                            if kt >= 4 * qc:
                                pat = kt - 4 * qc
                                off = 384 - pat * P
                                nc.vector.tensor_mul(
                                    out=ptile[:, j * CW:(j + 1) * CW],
                                    in0=ptile[:, j * CW:(j + 1) * CW],
                                    in1=maskr[:, off:off + CW])
                        for j, kt in enumerate(g):
                            nc.tensor.matmul(
                                out=pa[0:VW, :],
                                lhsT=vpad[:, kt, h * VW:(h + 1) * VW],
                                rhs=ptile[:, j * CW:(j + 1) * CW],
                                start=(kt == 0), stop=(kt == kmax - 1))

                    for g in groups:
                        pt = ps3.tile([P, GRP * CW], F32, tag="ps3",
                                      name=f"sc_{qc}_{h}_{g[0]}")
                        for j, kt in enumerate(g):
                            nc.tensor.matmul(
                                out=pt[:, j * CW:(j + 1) * CW],
                                lhsT=k_all[half:half + 64, h // 2,
                                           kt * P:(kt + 1) * P],
                                rhs=qs, start=True, stop=True)
                        if pending is not None:
                            flush(pending)
                        w = len(g) * CW
                        ptile = ptp.tile([P, GRP * CW], BF16, tag="pt",
                                         name=f"pt_{qc}_{h}_{g[0]}")
                        nc.scalar.activation(ptile[:, :w], pt[:, :w],
                                             AF.Exp, scale=0.125)
                        pending = (g, ptile)
                        pop_filler()
                    flush(pending)

                    # normalize by the ones-row denominator. The broadcast of
                    # the per-column denominator across partitions rides the
                    # PE (rank-1 matmul into pa's unused partitions 64..127)
                    # instead of GPSIMD: the in-order GPSIMD queue carries
                    # collective triggers and conversion DMAs whose waits
                    # would stall a latency-critical broadcast behind them.
                    den = denp.tile([1, CW], BF16, tag="den", name=f"den_{qc}_{h}")
                    nc.vector.tensor_copy(out=den[:], in_=pa[64:65, :])
                    nc.tensor.matmul(out=pa[64:P, :], lhsT=ones64[:],
                                     rhs=den[:], start=True, stop=True)
                    bcd = smallp.tile([64, CW], F32, tag="bcden",
                                      name=f"bcd_{qc}_{h}")
                    nc.vector.tensor_copy(out=bcd[:], in_=pa[64:P, :])
                    rc = smallp.tile([64, CW], F32, tag="recip",
                                     name=f"rc_{qc}_{h}")
                    nc.vector.reciprocal_approx_fast(out=rc[:], in_=bcd[:])
                    if debug:
                        nc.sync.dma_start(
                            out=dbg_rc[qc * HPC + h:qc * HPC + h + 1, :],
                            in_=rc[0:1, :])
                    if h % 2 == 0:
                        at_tiles[h // 2] = atp.tile(
                            [P, CW], BF16, tag=f"at{h // 2}",
                            name=f"at_{qc}_{h // 2}")
                    nc.vector.tensor_tensor(
                        out=at_tiles[h // 2][half:half + 64, :],
                        in0=pa[0:64, :], in1=rc[:], op=ALU.mult)

            # ---- main schedule ----
            # chunk 0 QKV up front; later chunks' QKV and deferred
            # projections ride as PE fillers inside attention, paced so
            # every chunk (especially the long last one) stays PE-dense.
            for u in qkv_units(0):
                u()
            for qc in range(NQC):
                if ablate == "qkv":
                    if qc + 1 < NQC:
                        for u in qkv_units(qc + 1):
                            u()
                    continue
                if ablate != "attn":
                    if qc == 2:
                        fillers.extend(proj_units(0))
                    if qc == 3:
                        fillers.extend(proj_units(1))
                        fillers.extend(proj_units(2))
                if qc + 1 < NQC:
                    fillers.extend(qkv_units(qc + 1))
                emit_attention(qc)
                drain_fillers()
            if ablate is None:
                # all output conversions at the tail: their RS inputs are
                # long done (0-2) so they never block the GPSIMD queue
                # mid-kernel; rs3 triggers right after conv0's cheap issue
                for u in conv_units(0):
                    u()
                for u in proj_units(NQC - 1):
                    u()
                for j in (1, 2, 3):
                    for u in conv_units(j):
                        u()
            if debug:
                for qc in range(NQC):
                    for m in range(4):
                        nc.sync.dma_start(
                            out=dbg_qk[:, m, qc * CW:(qc + 1) * CW],
                            in_=qt_tiles[qc][:, m, :])
                for m in range(4):
                    nc.sync.dma_start(out=dbg_qk[:, 4 + m, :],
                                      in_=k_all[:, m, :])
                for st in range(NKT):
                    nc.sync.dma_start(out=dbg_vp[:, st, :], in_=vpad[:, st, :])

    nc.finalize()
    return nc


def _get_nc():
    if "nc" not in _CACHE:
        _CACHE["nc"] = _build()
    return _CACHE["nc"]


def _make_mask():
    # M[p, c] = 1.0 iff (c - 384) >= p; pattern pat slice = cols [384-128*pat:][:CW]
    c = np.arange(MW)[None, :]
    p = np.arange(P)[:, None]
    return ((c - 384) >= p).astype(ml_dtypes.bfloat16)


def make_in_maps(x, W_attn, b_attn, W_proj, b_proj):
    x = np.asarray(x, np.float32)
    W_attn = np.asarray(W_attn, np.float32)
    b_attn = np.asarray(b_attn, np.float32)
    W_proj = np.asarray(W_proj, np.float32)
    b_proj = np.asarray(b_proj, np.float32)
    mask = _make_mask()
    in_maps = []
    for c in range(8):
        b, g = c // 2, c % 2
        sl = slice(g * DL, (g + 1) * DL)
        wqkv_c = np.concatenate([W_attn[:, g * DL:(g + 1) * DL],
                                 W_attn[:, D + g * DL:D + (g + 1) * DL],
                                 W_attn[:, 2 * D + g * DL:2 * D + (g + 1) * DL]],
                                axis=1)
        bqk_c = np.concatenate([b_attn[g * DL:(g + 1) * DL],
                                b_attn[D + g * DL:D + (g + 1) * DL]])
        in_maps.append({
            "xT": np.ascontiguousarray(x[b].T),
            "wqkv": np.ascontiguousarray(wqkv_c),
            "bqk": np.ascontiguousarray(bqk_c.reshape(8, P).T),
            "bv": b_attn[2 * D + g * DL:2 * D + (g + 1) * DL].reshape(1, DL).copy(),
            "wp": np.ascontiguousarray(W_proj[sl, :]).astype(ml_dtypes.bfloat16),
            "bp": (np.ascontiguousarray(b_proj.reshape(8, P).T) if g == 0
                   else np.zeros((P, 8), np.float32)),
            "maskc": mask,
        })
    return in_maps


def assemble(results):
    out = np.empty((B, S, D), np.float32)
    for c in range(8):
        b, g = c // 2, c % 2
        out[b][:, g * DL:(g + 1) * DL] = results[c]["out"].T
    return out


def kernel(x, W_attn, b_attn, W_proj, b_proj):
    from concourse.bass_utils import run_bass_kernel_spmd
    nc = _get_nc()
    in_maps = make_in_maps(x, W_attn, b_attn, W_proj, b_proj)
    res = run_bass_kernel_spmd(nc, in_maps, core_ids=list(range(8)))
    return assemble(res.results)
